# revision 10
# baseline (speedup 1.0000x reference)
"""GCN layer (dense projection + sparse neighbor aggregation) on 8 Trainium2
NeuronCores via Bass/Tile.

Strategy: shard nodes (and their incident edges, grouped by destination row)
across the 8 cores; replicate W/b; AllGather the projected node features so
every core can gather arbitrary source columns (4 bucket-aligned
sub-collectives pipelined with a bf16 projection); per 128-row output block,
bulk-gather the needed source rows with DMAGatherAnt (int16 indices into 4
sub-tables of <=32k rows), scale by edge_val, and segment-sum via an
assignment-matrix matmul accumulated in PSUM (bias pre-loaded into PSUM;
padded gather slots are killed by rowloc=-1).
"""

import sys

if "/opt/trn_rl_repo" not in sys.path:
    sys.path.insert(0, "/opt/trn_rl_repo")

import numpy as np

import concourse.bass as bass
import concourse.mybir as mybir
import concourse.tile as tile
from concourse import bacc
from concourse.bass_utils import run_bass_kernel_spmd

N_NODES = 100000
N_EDGES = 1600000
IN_FT = 256
OUT_FT = 64
NCORES = 8
NS = N_NODES // NCORES          # 12500 nodes per core
NB = (NS + 127) // 128          # 98 row blocks per core
NSP = NB * 128                  # 12544 padded nodes per core
GB = 7                          # row blocks per pipeline group (98 = 14 * 7)
NGROUPS = NB // GB              # 14
NSUB = 4                        # source-block buckets (int16 idx: <=32k rows)
QB = [25, 25, 24, 24]           # source blocks per bucket (sums to NB)
QBS = np.concatenate([[0], np.cumsum(QB)]).astype(np.int64)
SUBROWS = [NCORES * 128 * q for q in QB]

F32 = mybir.dt.float32
F16 = mybir.dt.float16
BF16 = mybir.dt.bfloat16
I32 = mybir.dt.int32
I16 = mybir.dt.int16

MAXCH = 8                       # 1024 indices = HW cap per dma_gather
NQ = 4                          # SWDGE queues (ucode max)


def build_program(nchb):
    """One SPMD Bass program; all 8 cores run it on their own shards.

    nchb[b]: 128-edge chunks per (row-block, bucket b).
    """
    nchb = list(nchb)
    ncht = sum(nchb)                    # chunks per row block
    off_b = np.concatenate([[0], np.cumsum(nchb)])  # chunk offset per bucket
    # per-(group,bucket) stream: GB*nchb[b] chunks; xg region offsets
    reg_b = np.concatenate([[0], np.cumsum([GB * c for c in nchb])])
    sgt = int(reg_b[-1])                # chunks per group in xg
    # gidx column offsets per bucket (16-wrapped: 8 int16 cols per chunk)
    gcol_b = np.concatenate([[0], np.cumsum([GB * c * 8 for c in nchb])])
    gcols = int(gcol_b[-1])

    nc = bacc.Bacc("TRN2", target_bir_lowering=False, debug=False,
                   num_devices=NCORES, num_swdge_queues=NQ)

    seqT = nc.dram_tensor("seqT", [2, 128, NSP], BF16, kind="ExternalInput")
    gidx = nc.dram_tensor("gidx", [128, NGROUPS, gcols], I16,
                          kind="ExternalInput")
    val = nc.dram_tensor("val", [128, NGROUPS, sgt], F16,
                         kind="ExternalInput")
    rl = nc.dram_tensor("rl", [128, NB, ncht], F16, kind="ExternalInput")
    w_in = nc.dram_tensor("w", [128, 2, OUT_FT], BF16, kind="ExternalInput")
    bias_in = nc.dram_tensor("bias128", [128, OUT_FT], F16,
                             kind="ExternalInput")
    iota_in = nc.dram_tensor("iotat", [128, ncht * 128], F16,
                             kind="ExternalInput")
    # partition-major layouts; host un-permutes
    sf_out = nc.dram_tensor("sf", [128, NB, OUT_FT], F32,
                            kind="ExternalOutput")
    ccin = [nc.dram_tensor(f"ccin{b}", [128, QB[b], OUT_FT], F32)
            for b in range(NSUB)]
    agg_out = nc.dram_tensor("agg", [128, NB, OUT_FT], F16,
                             kind="ExternalOutput")
    xfull = [nc.dram_tensor(f"xfull{b}", [SUBROWS[b], OUT_FT], F32,
                            addr_space="Shared") for b in range(NSUB)]

    groups = [list(range(NCORES))]

    with tile.TileContext(nc) as tc:
        with (
            tc.tile_pool(name="const", bufs=1) as cpool,
            tc.tile_pool(name="psum", bufs=4, space="PSUM") as psum_pool,
            tc.tile_pool(name="seqpan", bufs=2) as seqpan,
            tc.tile_pool(name="xbuck", bufs=2) as xbuck,
            tc.tile_pool(name="meta", bufs=2) as meta,
            tc.tile_pool(name="xgp", bufs=2) as xgp,
            tc.tile_pool(name="xg16p", bufs=2) as xg16p,
            tc.tile_pool(name="ohp", bufs=3) as ohp,
            tc.tile_pool(name="outp", bufs=2) as outp,
        ):
            w_sb = cpool.tile([128, 2, OUT_FT], BF16)
            nc.sync.dma_start(out=w_sb[:], in_=w_in[:])
            # bias/128 broadcast, fp16: added into PSUM via a ones matmul
            bias_sb = cpool.tile([128, OUT_FT], F16)
            nc.sync.dma_start(out=bias_sb[:], in_=bias_in[:])
            ones_sb = cpool.tile([128, 128], F16)
            nc.gpsimd.memset(ones_sb[:], 1.0)
            iota_sb = cpool.tile([128, ncht * 128], F16)
            nc.sync.dma_start(out=iota_sb[:], in_=iota_in[:])

            # ---- phase 1: x = seq @ W (bf16 -> f32 psum) + AllGathers ----
            for b in range(NSUB):
                pan = seqpan.tile([128, 2, QB[b] * 128], BF16, tag="pan")
                for kc in range(2):
                    nc.sync.dma_start(
                        out=pan[:, kc, :],
                        in_=seqT[kc, :, QBS[b] * 128:QBS[b + 1] * 128])
                xb = xbuck.tile([128, QB[b], OUT_FT], F32, tag="xb")
                for j in range(QB[b]):
                    px = psum_pool.tile([128, OUT_FT], F32, tag="px")
                    for kc in range(2):
                        nc.tensor.matmul(
                            px[:],
                            pan[:, kc, j * 128:(j + 1) * 128],
                            w_sb[:, kc, :],
                            start=(kc == 0),
                            stop=(kc == 1),
                        )
                    nc.vector.tensor_copy(out=xb[:, j, :], in_=px[:])
                nc.sync.dma_start(out=ccin[b][:], in_=xb[:])
                nc.sync.dma_start(
                    out=sf_out[:, QBS[b]:QBS[b + 1], :], in_=xb[:])
                nc.gpsimd.collective_compute(
                    "AllGather",
                    mybir.AluOpType.bypass,
                    replica_groups=groups,
                    ins=[ccin[b][:]],
                    outs=[xfull[b][:]],
                )

            # ---- phase 2: bulk gather + scale + segment-sum matmul ----
            for g in range(NGROUPS):
                gidx_sb = meta.tile([128, gcols], I16, tag="gidx")
                nc.sync.dma_start(out=gidx_sb[:], in_=gidx[:, g])
                val_sb = meta.tile([128, sgt], F16, tag="val")
                nc.sync.dma_start(out=val_sb[:], in_=val[:, g])
                rl_sb = meta.tile([128, GB, ncht], F16, tag="rl")
                nc.sync.dma_start(out=rl_sb[:],
                                  in_=rl[:, g * GB:(g + 1) * GB, :])
                # xg chunk layout per group: bucket-major regions;
                # bucket b block j chunk cc at reg_b[b] + j*nchb[b] + cc
                xg = xgp.tile([128, sgt, OUT_FT], F32, tag="xg")
                for b in range(NSUB):
                    sgb = GB * nchb[b]
                    for off in range(0, sgb, MAXCH):
                        ln = min(MAXCH, sgb - off)
                        r0 = int(reg_b[b]) + off
                        c0 = int(gcol_b[b]) + off * 8
                        nc.gpsimd.dma_gather(
                            out_ap=xg[:, r0:r0 + ln, :],
                            in_ap=xfull[b][:],
                            idxs_ap=gidx_sb[:, c0:c0 + ln * 8],
                            num_idxs=ln * 128,
                            num_idxs_reg=ln * 128,
                            elem_size=OUT_FT,
                            queue_num=b % NQ,
                        )
                # fold edge_val in while casting f32 -> fp16, per bucket
                # region so early buckets don't wait on late collectives
                xg16 = xg16p.tile([128, sgt, OUT_FT], F16, tag="xg16")
                for b in range(NSUB):
                    r0, r1 = int(reg_b[b]), int(reg_b[b + 1])
                    nc.vector.tensor_tensor(
                        out=xg16[:, r0:r1, :],
                        in0=xg[:, r0:r1, :],
                        in1=val_sb[:, r0:r1].unsqueeze(2).broadcast_to(
                            [128, r1 - r0, OUT_FT]),
                        op=mybir.AluOpType.mult,
                    )
                out_sb = outp.tile([128, GB, OUT_FT], F16, tag="out_sb")
                for j in range(GB):
                    # expand rowloc along q on the (otherwise idle) scalar
                    # engine, then a unit-stride is_equal on vector:
                    # A[p, c, q] = (rowloc[p, c] == q); -1 pads vanish
                    rlx = ohp.tile([128, ncht * 128], F16, tag="rlx")
                    nc.scalar.activation(
                        out=rlx[:].rearrange("p (c q) -> p c q", q=128),
                        in_=rl_sb[:, j, :].unsqueeze(2).broadcast_to(
                            [128, ncht, 128]),
                        func=mybir.ActivationFunctionType.Copy,
                    )
                    a_sb = ohp.tile([128, ncht * 128], F16, tag="a_sb")
                    nc.vector.tensor_tensor(
                        out=a_sb[:],
                        in0=rlx[:],
                        in1=iota_sb[:],
                        op=mybir.AluOpType.is_equal,
                    )
                    po = psum_pool.tile([128, OUT_FT], F32, tag="po")
                    nc.tensor.matmul(po[:], ones_sb[:], bias_sb[:],
                                     start=True, stop=False)
                    for b in range(NSUB):
                        for cc in range(nchb[b]):
                            ci = int(off_b[b]) + cc
                            rc = int(reg_b[b]) + j * nchb[b] + cc
                            nc.tensor.matmul(
                                po[:],
                                a_sb[:, ci * 128:(ci + 1) * 128],
                                xg16[:, rc, :],
                                start=False,
                                stop=(ci == ncht - 1),
                            )
                    nc.scalar.activation(
                        out=out_sb[:, j, :], in_=po[:],
                        func=mybir.ActivationFunctionType.Relu)
                nc.sync.dma_start(
                    out=agg_out[:, g * GB:(g + 1) * GB, :], in_=out_sb[:])

    nc.compile()
    return nc


def prepare_inputs(seq, edge_row, edge_col, edge_val, W, b):
    """Host-side sharding / graph partitioning. Returns (in_maps, nchb)."""
    import ml_dtypes

    seq = np.asarray(seq, dtype=np.float32).reshape(N_NODES, IN_FT)
    r = np.asarray(edge_row).astype(np.int64)
    c = np.asarray(edge_col).astype(np.int64)
    v = np.asarray(edge_val, dtype=np.float32)
    W = np.asarray(W, dtype=np.float32).reshape(IN_FT, OUT_FT)
    b = np.asarray(b, dtype=np.float32).reshape(OUT_FT)

    # bucket (by source block) of each block index
    blk_q = np.searchsorted(QBS[1:], np.arange(NB), side="right")
    qb_arr = np.asarray(QB)

    # feature-table row within its bucket sub-table
    csrc = c // NS
    crem = c % NS
    cblk = crem // 128
    cp = crem % 128
    cq = blk_q[cblk]
    lidx = ((csrc * 128 + cp) * qb_arr[cq] + (cblk - QBS[cq])).astype(
        np.int16)

    core = r // NS
    loc = r - core * NS
    blk = loc >> 7
    rowloc = (loc & 127).astype(np.float16)
    bucket = cq

    # per-bucket chunk capacity from per-(core, block, bucket) degrees
    key = (core * NB + blk) * NSUB + bucket
    ngrp = NCORES * NB * NSUB
    counts = np.bincount(key, minlength=ngrp).reshape(NCORES, NB, NSUB)
    nchb = [max(1, int(np.ceil(counts[:, :, b_].max() / 128)))
            for b_ in range(NSUB)]
    caps = np.array([c_ * 128 for c_ in nchb])
    off_edge = np.concatenate([[0], np.cumsum(caps)])  # within (core, block)
    tot_cap = int(off_edge[-1])                        # ncht * 128

    order = np.argsort(key, kind="stable")
    key_s = key[order]
    starts = np.searchsorted(key_s, np.arange(ngrp))
    pos = np.arange(N_EDGES) - starts[key_s]
    kb = key_s % NSUB
    kcb = key_s // NSUB          # core * NB + blk
    dest = kcb * tot_cap + off_edge[kb] + pos

    idxp = np.zeros(NCORES * NB * tot_cap, np.int16)       # pad: row 0
    valp = np.zeros(NCORES * NB * tot_cap, np.float16)
    rlp = np.full(NCORES * NB * tot_cap, -1.0, np.float16)  # pad: killed
    idxp[dest] = lidx[order]
    valp[dest] = v[order].astype(np.float16)
    rlp[dest] = rowloc[order]

    # [core, block, chunk(b,cc), lane] views
    idxp = idxp.reshape(NCORES, NB, tot_cap)
    valp = valp.reshape(NCORES, NB, tot_cap)
    rlp = rlp.reshape(NCORES, NB, tot_cap)

    # rl: block-major [core, 128, NB, ncht]
    ncht = sum(nchb)
    rl_l = rlp.reshape(NCORES, NB, ncht, 128).transpose(0, 3, 1, 2)
    rl_arr = np.ascontiguousarray(rl_l)

    # val + gidx: per (group, bucket) streams (blocks of the group concat)
    sgt = GB * ncht
    val_arr = np.empty((NCORES, 128, NGROUPS, sgt), np.float16)
    gcols = sgt * 8
    gidx_arr = np.empty((NCORES, 16, NGROUPS, gcols), np.int16)
    reg0 = 0
    gc0 = 0
    for b_ in range(NSUB):
        cb = caps[b_]
        sgb = GB * nchb[b_]
        e0, e1 = off_edge[b_], off_edge[b_ + 1]
        # [core, group, GB, cb] -> stream [core, group, GB*cb]
        seg_i = idxp[:, :, e0:e1].reshape(NCORES, NGROUPS, GB * cb)
        seg_v = valp[:, :, e0:e1].reshape(NCORES, NGROUPS, GB * cb)
        # val chunk-lane layout [128, chunks]
        vl = seg_v.reshape(NCORES, NGROUPS, sgb, 128).transpose(0, 3, 1, 2)
        val_arr[:, :, :, reg0:reg0 + sgb] = vl
        # idx 16-wrap: i -> [i % 16, i // 16]
        wi = seg_i.reshape(NCORES, NGROUPS, GB * cb // 16, 16)
        gidx_arr[:, :, :, gc0:gc0 + GB * cb // 16] = wi.transpose(0, 3, 1, 2)
        reg0 += sgb
        gc0 += GB * cb // 16
    gidx_full = np.broadcast_to(
        gidx_arr[:, None], (NCORES, 8, 16, NGROUPS, gcols))
    gidx_full = np.ascontiguousarray(
        gidx_full.reshape(NCORES, 128, NGROUPS, gcols))

    bias128 = np.broadcast_to((b / 128.0).astype(np.float16),
                              (128, OUT_FT)).copy()
    iotat = np.broadcast_to(
        np.tile(np.arange(128, dtype=np.float16), ncht),
        (128, ncht * 128)).copy()
    w3 = np.ascontiguousarray(
        W.reshape(2, 128, OUT_FT).transpose(1, 0, 2)).astype(
            ml_dtypes.bfloat16)  # [128, 2, OUT_FT]

    in_maps = []
    for k in range(NCORES):
        shard = np.zeros((NSP, IN_FT), np.float32)
        shard[:NS] = seq[k * NS:(k + 1) * NS]
        seqT_k = np.ascontiguousarray(shard.T).reshape(
            2, 128, NSP).astype(ml_dtypes.bfloat16)
        in_maps.append({
            "seqT": seqT_k,
            "gidx": gidx_full[k],
            "val": np.ascontiguousarray(val_arr[k]),
            "rl": rl_arr[k],
            "w": w3,
            "bias128": bias128,
            "iotat": iotat,
        })
    return in_maps, tuple(nchb)


_PROGRAMS: dict[tuple, object] = {}


def kernel(seq, edge_row, edge_col, edge_val, W, b):
    in_maps, nchb = prepare_inputs(seq, edge_row, edge_col, edge_val, W, b)
    prog = _PROGRAMS.get(nchb)
    if prog is None:
        prog = _PROGRAMS[nchb] = build_program(nchb)
    res = run_bass_kernel_spmd(prog, in_maps, core_ids=list(range(NCORES)))

    def unshard_agg():
        parts = [
            np.asarray(res.results[k]["agg"], dtype=np.float32)
            .transpose(1, 0, 2).reshape(NSP, OUT_FT)[:NS]
            for k in range(NCORES)
        ]
        return np.concatenate(parts)[None]

    def unshard_sf():
        parts = [
            np.asarray(res.results[k]["sf"])
            .transpose(1, 0, 2).reshape(NSP, OUT_FT)[:NS]
            for k in range(NCORES)
        ]
        return np.concatenate(parts)[None].astype(np.float32)

    return unshard_agg(), unshard_sf()


# revision 13
# speedup vs baseline: 1.0230x; 1.0230x over previous
"""GCN layer (dense projection + sparse neighbor aggregation) on 8 Trainium2
NeuronCores via Bass/Tile.

Strategy: shard nodes (and their incident edges, grouped by destination row)
across the 8 cores; replicate W/b; AllGather the projected node features so
every core can gather arbitrary source columns (4 bucket-aligned
sub-collectives pipelined with a bf16 projection); per 128-row output block,
bulk-gather the needed source rows with DMAGatherAnt (int16 indices into 4
sub-tables of <=32k rows), scale by edge_val, and segment-sum via an
assignment-matrix matmul accumulated in PSUM (bias pre-loaded into PSUM;
padded gather slots are killed by rowloc=-1).
"""

import sys

if "/opt/trn_rl_repo" not in sys.path:
    sys.path.insert(0, "/opt/trn_rl_repo")

import numpy as np

import concourse.bass as bass
import concourse.mybir as mybir
import concourse.tile as tile
from concourse import bacc
from concourse.bass_utils import run_bass_kernel_spmd

N_NODES = 100000
N_EDGES = 1600000
IN_FT = 256
OUT_FT = 64
NCORES = 8
NS = N_NODES // NCORES          # 12500 nodes per core
NB = (NS + 127) // 128          # 98 row blocks per core
NSP = NB * 128                  # 12544 padded nodes per core
GB = 7                          # row blocks per pipeline group (98 = 14 * 7)
NGROUPS = NB // GB              # 14
NSUB = 4                        # source-block buckets (int16 idx: <=32k rows)
QB = [25, 25, 24, 24]           # source blocks per bucket (sums to NB)
QBS = np.concatenate([[0], np.cumsum(QB)]).astype(np.int64)
SUBROWS = [NCORES * 128 * q for q in QB]

F32 = mybir.dt.float32
F16 = mybir.dt.float16
BF16 = mybir.dt.bfloat16
I32 = mybir.dt.int32
I16 = mybir.dt.int16

MAXCH = 8                       # 1024 indices = HW cap per dma_gather
NQ = 4                          # SWDGE queues (ucode max)


def build_program(nchb):
    """One SPMD Bass program; all 8 cores run it on their own shards.

    nchb[b]: 128-edge chunks per (row-block, bucket b).
    """
    nchb = list(nchb)
    ncht = sum(nchb)                    # chunks per row block
    off_b = np.concatenate([[0], np.cumsum(nchb)])  # chunk offset per bucket
    # per-(group,bucket) stream: GB*nchb[b] chunks; xg region offsets
    reg_b = np.concatenate([[0], np.cumsum([GB * c for c in nchb])])
    sgt = int(reg_b[-1])                # chunks per group in xg
    # gidx column offsets per bucket (16-wrapped: 8 int16 cols per chunk)
    gcol_b = np.concatenate([[0], np.cumsum([GB * c * 8 for c in nchb])])
    gcols = int(gcol_b[-1])

    nc = bacc.Bacc("TRN2", target_bir_lowering=False, debug=False,
                   num_devices=NCORES, num_swdge_queues=NQ)

    seqT = nc.dram_tensor("seqT", [2, 128, NSP], BF16, kind="ExternalInput")
    gidx = nc.dram_tensor("gidx", [128, NGROUPS, gcols], I16,
                          kind="ExternalInput")
    val = nc.dram_tensor("val", [128, NGROUPS, sgt], F16,
                         kind="ExternalInput")
    rl = nc.dram_tensor("rl", [128, NB, ncht], F16, kind="ExternalInput")
    w_in = nc.dram_tensor("w", [128, 2, OUT_FT], BF16, kind="ExternalInput")
    bias_in = nc.dram_tensor("bias128", [128, OUT_FT], F16,
                             kind="ExternalInput")
    iota_in = nc.dram_tensor("iotat", [128, ncht * 128], F16,
                             kind="ExternalInput")
    # partition-major layouts; host un-permutes
    sf_out = nc.dram_tensor("sf", [128, NB, OUT_FT], F32,
                            kind="ExternalOutput")
    ccin = [nc.dram_tensor(f"ccin{b}", [128, QB[b], OUT_FT], F32)
            for b in range(NSUB)]
    agg_out = nc.dram_tensor("agg", [128, NB, OUT_FT], F16,
                             kind="ExternalOutput")
    xfull = [nc.dram_tensor(f"xfull{b}", [SUBROWS[b], OUT_FT], F32,
                            addr_space="Shared") for b in range(NSUB)]

    groups = [list(range(NCORES))]

    with tile.TileContext(nc) as tc:
        with (
            tc.tile_pool(name="const", bufs=1) as cpool,
            tc.tile_pool(name="psum", bufs=4, space="PSUM") as psum_pool,
        ):
            w_sb = cpool.tile([128, 2, OUT_FT], BF16)
            nc.sync.dma_start(out=w_sb[:], in_=w_in[:])
            # bias/128 broadcast, fp16: added into PSUM via a ones matmul
            bias_sb = cpool.tile([128, OUT_FT], F16)
            nc.sync.dma_start(out=bias_sb[:], in_=bias_in[:])
            ones_sb = cpool.tile([128, 128], F16)
            nc.gpsimd.memset(ones_sb[:], 1.0)
            iota_sb = cpool.tile([128, ncht * 128], F16)
            nc.sync.dma_start(out=iota_sb[:], in_=iota_in[:])

            # ---- phase 1: x = seq @ W (bf16 -> f32 psum) + AllGathers ----
            with (
                tc.tile_pool(name="seqpan", bufs=2) as seqpan,
                tc.tile_pool(name="xbuck", bufs=2) as xbuck,
            ):
                for b in range(NSUB):
                    pan = seqpan.tile([128, 2, QB[b] * 128], BF16, tag="pan")
                    for kc in range(2):
                        nc.sync.dma_start(
                            out=pan[:, kc, :],
                            in_=seqT[kc, :, QBS[b] * 128:QBS[b + 1] * 128])
                    xb = xbuck.tile([128, QB[b], OUT_FT], F32, tag="xb")
                    for j in range(QB[b]):
                        px = psum_pool.tile([128, OUT_FT], F32, tag="px")
                        for kc in range(2):
                            nc.tensor.matmul(
                                px[:],
                                pan[:, kc, j * 128:(j + 1) * 128],
                                w_sb[:, kc, :],
                                start=(kc == 0),
                                stop=(kc == 1),
                            )
                        nc.vector.tensor_copy(out=xb[:, j, :], in_=px[:])
                    nc.sync.dma_start(out=ccin[b][:], in_=xb[:])
                    nc.sync.dma_start(
                        out=sf_out[:, QBS[b]:QBS[b + 1], :], in_=xb[:])
                    nc.gpsimd.collective_compute(
                        "AllGather",
                        mybir.AluOpType.bypass,
                        replica_groups=groups,
                        ins=[ccin[b][:]],
                        outs=[xfull[b][:]],
                    )

            # ---- phase 2: bulk gather + scale + segment-sum matmul ----
            with (
                tc.tile_pool(name="meta", bufs=3) as meta,
                tc.tile_pool(name="xgp", bufs=3) as xgp,
                tc.tile_pool(name="xg16p", bufs=2) as xg16p,
                tc.tile_pool(name="ohp", bufs=3) as ohp,
                tc.tile_pool(name="outp", bufs=2) as outp,
            ):
                for g in range(NGROUPS):
                    gidx_sb = meta.tile([128, gcols], I16, tag="gidx")
                    nc.sync.dma_start(out=gidx_sb[:], in_=gidx[:, g])
                    val_sb = meta.tile([128, sgt], F16, tag="val")
                    nc.sync.dma_start(out=val_sb[:], in_=val[:, g])
                    rl_sb = meta.tile([128, GB, ncht], F16, tag="rl")
                    nc.sync.dma_start(out=rl_sb[:],
                                      in_=rl[:, g * GB:(g + 1) * GB, :])
                    # per-bucket gather tiles so bucket b of many groups can
                    # stream as soon as AllGather b lands; per-bucket scale
                    # frees each tile independently
                    xg16 = xg16p.tile([128, sgt, OUT_FT], F16, tag="xg16")
                    for b in range(NSUB):
                        sgb = GB * nchb[b]
                        xgb = xgp.tile([128, sgb, OUT_FT], F32,
                                       tag=f"xg{b}")
                        for off in range(0, sgb, MAXCH):
                            ln = min(MAXCH, sgb - off)
                            c0 = int(gcol_b[b]) + off * 8
                            nc.gpsimd.dma_gather(
                                out_ap=xgb[:, off:off + ln, :],
                                in_ap=xfull[b][:],
                                idxs_ap=gidx_sb[:, c0:c0 + ln * 8],
                                num_idxs=ln * 128,
                                num_idxs_reg=ln * 128,
                                elem_size=OUT_FT,
                                queue_num=b % NQ,
                            )
                        r0 = int(reg_b[b])
                        nc.vector.tensor_tensor(
                            out=xg16[:, r0:r0 + sgb, :],
                            in0=xgb[:],
                            in1=val_sb[:, r0:r0 + sgb].unsqueeze(2)
                            .broadcast_to([128, sgb, OUT_FT]),
                            op=mybir.AluOpType.mult,
                        )
                    out_sb = outp.tile([128, GB, OUT_FT], F16, tag="out_sb")
                    for j in range(GB):
                        # expand rowloc along q on the (otherwise idle)
                        # scalar engine, then a unit-stride is_equal on
                        # vector: A[p, c, q] = (rowloc[p, c] == q)
                        rlx = ohp.tile([128, ncht * 128], F16, tag="rlx")
                        nc.scalar.activation(
                            out=rlx[:].rearrange("p (c q) -> p c q", q=128),
                            in_=rl_sb[:, j, :].unsqueeze(2).broadcast_to(
                                [128, ncht, 128]),
                            func=mybir.ActivationFunctionType.Copy,
                        )
                        a_sb = ohp.tile([128, ncht * 128], F16, tag="a_sb")
                        nc.vector.tensor_tensor(
                            out=a_sb[:],
                            in0=rlx[:],
                            in1=iota_sb[:],
                            op=mybir.AluOpType.is_equal,
                        )
                        po = psum_pool.tile([128, OUT_FT], F32, tag="po")
                        nc.tensor.matmul(po[:], ones_sb[:], bias_sb[:],
                                         start=True, stop=False)
                        for b in range(NSUB):
                            for cc in range(nchb[b]):
                                ci = int(off_b[b]) + cc
                                rc = int(reg_b[b]) + j * nchb[b] + cc
                                nc.tensor.matmul(
                                    po[:],
                                    a_sb[:, ci * 128:(ci + 1) * 128],
                                    xg16[:, rc, :],
                                    start=False,
                                    stop=(ci == ncht - 1),
                                )
                        nc.scalar.activation(
                            out=out_sb[:, j, :], in_=po[:],
                            func=mybir.ActivationFunctionType.Relu)
                    nc.sync.dma_start(
                        out=agg_out[:, g * GB:(g + 1) * GB, :], in_=out_sb[:])

    nc.compile()
    return nc


def prepare_inputs(seq, edge_row, edge_col, edge_val, W, b):
    """Host-side sharding / graph partitioning. Returns (in_maps, nchb)."""
    import ml_dtypes

    seq = np.asarray(seq, dtype=np.float32).reshape(N_NODES, IN_FT)
    r = np.asarray(edge_row).astype(np.int64)
    c = np.asarray(edge_col).astype(np.int64)
    v = np.asarray(edge_val, dtype=np.float32)
    W = np.asarray(W, dtype=np.float32).reshape(IN_FT, OUT_FT)
    b = np.asarray(b, dtype=np.float32).reshape(OUT_FT)

    # bucket (by source block) of each block index
    blk_q = np.searchsorted(QBS[1:], np.arange(NB), side="right")
    qb_arr = np.asarray(QB)

    # feature-table row within its bucket sub-table
    csrc = c // NS
    crem = c % NS
    cblk = crem // 128
    cp = crem % 128
    cq = blk_q[cblk]
    lidx = ((csrc * 128 + cp) * qb_arr[cq] + (cblk - QBS[cq])).astype(
        np.int16)

    core = r // NS
    loc = r - core * NS
    blk = loc >> 7
    rowloc = (loc & 127).astype(np.float16)
    bucket = cq

    # per-bucket chunk capacity from per-(core, block, bucket) degrees
    key = (core * NB + blk) * NSUB + bucket
    ngrp = NCORES * NB * NSUB
    counts = np.bincount(key, minlength=ngrp).reshape(NCORES, NB, NSUB)
    nchb = [max(1, int(np.ceil(counts[:, :, b_].max() / 128)))
            for b_ in range(NSUB)]
    caps = np.array([c_ * 128 for c_ in nchb])
    off_edge = np.concatenate([[0], np.cumsum(caps)])  # within (core, block)
    tot_cap = int(off_edge[-1])                        # ncht * 128

    order = np.argsort(key, kind="stable")
    key_s = key[order]
    starts = np.searchsorted(key_s, np.arange(ngrp))
    pos = np.arange(N_EDGES) - starts[key_s]
    kb = key_s % NSUB
    kcb = key_s // NSUB          # core * NB + blk
    dest = kcb * tot_cap + off_edge[kb] + pos

    idxp = np.zeros(NCORES * NB * tot_cap, np.int16)       # pad: row 0
    valp = np.zeros(NCORES * NB * tot_cap, np.float16)
    rlp = np.full(NCORES * NB * tot_cap, -1.0, np.float16)  # pad: killed
    idxp[dest] = lidx[order]
    valp[dest] = v[order].astype(np.float16)
    rlp[dest] = rowloc[order]

    # [core, block, chunk(b,cc), lane] views
    idxp = idxp.reshape(NCORES, NB, tot_cap)
    valp = valp.reshape(NCORES, NB, tot_cap)
    rlp = rlp.reshape(NCORES, NB, tot_cap)

    # rl: block-major [core, 128, NB, ncht]
    ncht = sum(nchb)
    rl_l = rlp.reshape(NCORES, NB, ncht, 128).transpose(0, 3, 1, 2)
    rl_arr = np.ascontiguousarray(rl_l)

    # val + gidx: per (group, bucket) streams (blocks of the group concat)
    sgt = GB * ncht
    val_arr = np.empty((NCORES, 128, NGROUPS, sgt), np.float16)
    gcols = sgt * 8
    gidx_arr = np.empty((NCORES, 16, NGROUPS, gcols), np.int16)
    reg0 = 0
    gc0 = 0
    for b_ in range(NSUB):
        cb = caps[b_]
        sgb = GB * nchb[b_]
        e0, e1 = off_edge[b_], off_edge[b_ + 1]
        # [core, group, GB, cb] -> stream [core, group, GB*cb]
        seg_i = idxp[:, :, e0:e1].reshape(NCORES, NGROUPS, GB * cb)
        seg_v = valp[:, :, e0:e1].reshape(NCORES, NGROUPS, GB * cb)
        # val chunk-lane layout [128, chunks]
        vl = seg_v.reshape(NCORES, NGROUPS, sgb, 128).transpose(0, 3, 1, 2)
        val_arr[:, :, :, reg0:reg0 + sgb] = vl
        # idx 16-wrap: i -> [i % 16, i // 16]
        wi = seg_i.reshape(NCORES, NGROUPS, GB * cb // 16, 16)
        gidx_arr[:, :, :, gc0:gc0 + GB * cb // 16] = wi.transpose(0, 3, 1, 2)
        reg0 += sgb
        gc0 += GB * cb // 16
    gidx_full = np.broadcast_to(
        gidx_arr[:, None], (NCORES, 8, 16, NGROUPS, gcols))
    gidx_full = np.ascontiguousarray(
        gidx_full.reshape(NCORES, 128, NGROUPS, gcols))

    bias128 = np.broadcast_to((b / 128.0).astype(np.float16),
                              (128, OUT_FT)).copy()
    iotat = np.broadcast_to(
        np.tile(np.arange(128, dtype=np.float16), ncht),
        (128, ncht * 128)).copy()
    w3 = np.ascontiguousarray(
        W.reshape(2, 128, OUT_FT).transpose(1, 0, 2)).astype(
            ml_dtypes.bfloat16)  # [128, 2, OUT_FT]

    in_maps = []
    for k in range(NCORES):
        shard = np.zeros((NSP, IN_FT), np.float32)
        shard[:NS] = seq[k * NS:(k + 1) * NS]
        seqT_k = np.ascontiguousarray(shard.T).reshape(
            2, 128, NSP).astype(ml_dtypes.bfloat16)
        in_maps.append({
            "seqT": seqT_k,
            "gidx": gidx_full[k],
            "val": np.ascontiguousarray(val_arr[k]),
            "rl": rl_arr[k],
            "w": w3,
            "bias128": bias128,
            "iotat": iotat,
        })
    return in_maps, tuple(nchb)


_PROGRAMS: dict[tuple, object] = {}


def kernel(seq, edge_row, edge_col, edge_val, W, b):
    in_maps, nchb = prepare_inputs(seq, edge_row, edge_col, edge_val, W, b)
    prog = _PROGRAMS.get(nchb)
    if prog is None:
        prog = _PROGRAMS[nchb] = build_program(nchb)
    res = run_bass_kernel_spmd(prog, in_maps, core_ids=list(range(NCORES)))

    def unshard_agg():
        parts = [
            np.asarray(res.results[k]["agg"], dtype=np.float32)
            .transpose(1, 0, 2).reshape(NSP, OUT_FT)[:NS]
            for k in range(NCORES)
        ]
        return np.concatenate(parts)[None]

    def unshard_sf():
        parts = [
            np.asarray(res.results[k]["sf"])
            .transpose(1, 0, 2).reshape(NSP, OUT_FT)[:NS]
            for k in range(NCORES)
        ]
        return np.concatenate(parts)[None].astype(np.float32)

    return unshard_agg(), unshard_sf()
